# revision 13
# baseline (speedup 1.0000x reference)
"""Causal self-attention (B=4, T=2048, C=1024, H=16) on 8 TRN2 NeuronCores.

Sharding: core c = (b, hg) with b = c//2 batch index, hg = c%2 head-group
(8 heads each).  Each core computes its batch element's attention for its 8
heads plus the partial c_proj (W_proj column-shard); the host sums the two
head-group partials per batch element.

v2 pipeline (vs v1: stage-3 re-oriented so V is the stationary operand and
the 512-wide tq block streams, which removes the 1280 LDWEIGHTS-bound
65-col matmuls AND the output transposes; causal exp restricted to the
valid region with a 128x128 triangle mask instead of full-tile 0/1 masks):
  stage 1 (fp32r): qkT[j,t] = WqkT^T-contract(xT); V[t,jv] bf16 with a ones
                   column per head ([V_h | 1], 65 cols).
  stage 2 (fp32r): S.T[s,tq] = K lhsT vs Q rhs, two heads concurrent via
                   tile_position (0,0)/(64,0).
  exp (ACT):       P = exp(S.T/8) bf16; boundary s-tiles exp only the
                   causal cols, memset the dead strip, triangle-mask the
                   diagonal 128x128 block (DVE).
  stage 3 (bf16):  OT[65, tq512] = [V_h|1] lhsT vs P rhs, accumulated over
                   s-tiles; row 64 = softmax denominator.
  normalize:       gpsimd partition_broadcast denom row -> [64,512], DVE
                   reciprocal + mul -> yT bf16 (head b shifted to
                   partitions 64:128 by SBUF->SBUF DMA).
  stage 4 (bf16):  out[t,co] = yT lhsT vs WpT rhs, accumulate over j.

`reps` > 1 repeats the whole body inside one NEFF (for wall-clock timing by
differencing, since per-dispatch overhead through axon is ~70-90 ms).
"""
import numpy as np
import ml_dtypes

import concourse.bacc as bacc
import concourse.mybir as mybir
import concourse.tile as tile
from concourse.bass_utils import run_bass_kernel_spmd

F32 = mybir.dt.float32
F32R = mybir.dt.float32r
BF16 = mybir.dt.bfloat16

B, C, NH, HD = 4, 1024, 16, 64
HPC = 8              # heads per core
JV = HPC * HD        # 512: v-feature cols per core
KC = C // 128        # 8 contraction chunks
SCALE = 1.0 / 8.0    # 1/sqrt(HD)


def emit_body(nc, tc, dram, T):
    TT = T // 128
    TQB = T // 512
    xT, wqkT, wvT, wpTb, tri, yout = (
        dram["xT"], dram["wqkT"], dram["wvT"], dram["wpTb"],
        dram["tri"], dram["yout"])

    with tc.tile_pool(name="persist", bufs=1) as pers:
        qkT_sb = pers.tile([128, 8, T], F32R)          # [j-part, jc, t]
        vext_sb = pers.tile([128, TT, HPC, 65], BF16)  # [s-part, st, h, d|1]
        tri_sb = pers.tile([128, 128], BF16)           # tri[p,c]=1 iff c>=p
        selt = pers.tile([65, 64], F32R)               # ones row at p=64
        nc.sync.dma_start(tri_sb[:], tri[:])
        nc.sync.dma_start(selt[64:65, :], dram["ones64"][:])

        with tc.tile_pool(name="s2ps", bufs=2, space="PSUM") as s2ps, \
             tc.tile_pool(name="mmx", bufs=2, space="PSUM") as ps512, \
             tc.tile_pool(name="ps3p", bufs=2, space="PSUM") as ps3p:

            # ---------------- stage 1 ----------------
            with tc.tile_pool(name="stage1", bufs=1) as s1p:
                xT_sb = s1p.tile([128, KC, T], F32R)
                wqk_sb = s1p.tile([128, KC, 1024], F32R)
                wv_sb = s1p.tile([128, KC, JV], F32R)
                xT3 = xT.rearrange("(kc p) t -> p kc t", p=128)
                wqk3 = wqkT.rearrange("(kc p) j -> p kc j", p=128)
                wv3 = wvT.rearrange("(kc p) j -> p kc j", p=128)
                for kc in range(KC):
                    nc.sync.dma_start(xT_sb[:, kc, :], xT3[:, kc, :])
                    nc.sync.dma_start(wqk_sb[:, kc, :], wqk3[:, kc, :])
                    nc.sync.dma_start(wv_sb[:, kc, :], wv3[:, kc, :])

                # qkT = WqkT.T-contract(xT): out chunk jc over t blocks
                for jc in range(8):
                    for nb in range(TQB):
                        ps = ps512.tile([128, 512], F32, tag="ps512")
                        for kc in range(KC):
                            nc.tensor.matmul(
                                ps[:],
                                wqk_sb[:, kc, jc * 128:(jc + 1) * 128],
                                xT_sb[:, kc, nb * 512:(nb + 1) * 512],
                                start=(kc == 0), stop=(kc == KC - 1))
                        nc.vector.tensor_copy(
                            qkT_sb[:, jc, nb * 512:(nb + 1) * 512], ps[:])
                # V = xT.T-contract(WvT): out t-chunk tt, 512 v-cols
                for tt in range(TT):
                    ps = ps512.tile([128, 512], F32, tag="ps512")
                    for kc in range(KC):
                        nc.tensor.matmul(
                            ps[:],
                            xT_sb[:, kc, tt * 128:(tt + 1) * 128],
                            wv_sb[:, kc, :],
                            start=(kc == 0), stop=(kc == KC - 1))
                    nc.vector.tensor_copy(
                        vext_sb[:, tt, :, 0:64],
                        ps[:].rearrange("p (h d) -> p h d", h=HPC))
                    nc.vector.memset(vext_sb[:, tt, :, 64:65], 1.0)

            # ---------------- attention + proj ----------------
            with tc.tile_pool(name="wp", bufs=1) as wpp, \
                 tc.tile_pool(name="pexp", bufs=2) as ppool, \
                 tc.tile_pool(name="ytpool", bufs=2) as ytpool, \
                 tc.tile_pool(name="ybpool", bufs=2) as ybpool, \
                 tc.tile_pool(name="rbpool", bufs=4) as rbpool, \
                 tc.tile_pool(name="outp", bufs=3) as outp:
                wp_sb = wpp.tile([128, 4, C], BF16)
                wp3 = wpTb.rearrange("(jc p) co -> p jc co", p=128)
                for jc in range(4):
                    nc.sync.dma_start(wp_sb[:, jc, :], wp3[:, jc, :])

                for tqb in range(TQB):
                    nst = 4 * (tqb + 1)     # causal: s-tiles 0..nst-1
                    yT_t = ytpool.tile([128, 4, 512], BF16, tag="yt")
                    for pc in range(4):
                        pab = ppool.tile([128, TT, 1024], BF16, tag="pab")
                        qs = 2 * pc         # chunk with [Qa|Qb]
                        ks = 2 * pc + 1     # chunk with [Ka|Kb]
                        tqs = slice(tqb * 512, (tqb + 1) * 512)
                        for st in range(nst):
                            ss = slice(st * 128, (st + 1) * 128)
                            psAB = s2ps.tile([128, 1024], F32, tag="s2")
                            nc.tensor.matmul(
                                psAB[:, 0:512], qkT_sb[0:64, ks, ss],
                                qkT_sb[0:64, qs, tqs],
                                start=True, stop=True, tile_position=(0, 0))
                            nc.tensor.matmul(
                                psAB[:, 512:1024], qkT_sb[64:128, ks, ss],
                                qkT_sb[64:128, qs, tqs],
                                start=True, stop=True, tile_position=(64, 0))
                            q = st - 4 * tqb
                            if q < 0:       # fully-valid s-tile
                                nc.scalar.activation(
                                    pab[:, st, :], psAB[:],
                                    mybir.ActivationFunctionType.Exp,
                                    scale=SCALE)
                            else:           # boundary s-tile: causal edge
                                for hoff in (0, 1):
                                    base = hoff * 512
                                    if q > 0:
                                        nc.vector.memset(
                                            pab[:, st,
                                                base:base + q * 128], 0.0)
                                    nc.scalar.activation(
                                        pab[:, st, base + q * 128:base + 512],
                                        psAB[:, base + q * 128:base + 512],
                                        mybir.ActivationFunctionType.Exp,
                                        scale=SCALE)
                                    nc.vector.tensor_mul(
                                        pab[:, st,
                                            base + q * 128:base + (q + 1) * 128],
                                        pab[:, st,
                                            base + q * 128:base + (q + 1) * 128],
                                        tri_sb[:])
                        psOs = []
                        for hoff in (0, 1):
                            h = 2 * pc + hoff
                            psO = ps3p.tile([128, 512], F32, tag="s3")
                            for st in range(nst):
                                nc.tensor.matmul(
                                    psO[0:65, :],
                                    vext_sb[:, st, h, :],
                                    pab[:, st, hoff * 512:(hoff + 1) * 512],
                                    start=(st == 0), stop=(st == nst - 1))
                            psOs.append(psO)
                        # normalize: yT_h = OT_h * (1/denom_h); reciprocal of
                        # the denom row stays on partition 64 (lane-aligned),
                        # then a K=1 ones-row matmul broadcasts it across 64
                        # output partitions.
                        rcp2 = rbpool.tile([65, 2, 512], F32R, tag="rcp")
                        with nc.allow_low_precision(
                                reason="f32r is bit-identical to f32; "
                                       "needed for 1-cyc/row PE broadcast"):
                            nc.vector.reciprocal(
                                rcp2[64:65, 0, :], psOs[0][64:65, :])
                            nc.vector.reciprocal(
                                rcp2[64:65, 1, :], psOs[1][64:65, :])
                        psRa = s2ps.tile([128, 1024], F32, tag="s2")
                        nc.tensor.matmul(
                            psRa[0:64, 0:512], selt[64:65, :],
                            rcp2[64:65, 0, :], start=True, stop=True)
                        psRb = s2ps.tile([128, 1024], F32, tag="s2")
                        nc.tensor.matmul(
                            psRb[0:64, 0:512], selt[64:65, :],
                            rcp2[64:65, 1, :], start=True, stop=True)
                        rba = rbpool.tile([64, 512], F32, tag="rba")
                        nc.vector.tensor_copy(rba[:], psRa[0:64, 0:512])
                        rbb = rbpool.tile([64, 512], F32, tag="rbb")
                        nc.vector.tensor_copy(rbb[:], psRb[0:64, 0:512])
                        with nc.allow_low_precision(
                                reason="attn output feeds bf16 c_proj"):
                            nc.vector.tensor_mul(
                                yT_t[0:64, pc, :], psOs[0][0:64, :], rba[:])
                            ytb = ybpool.tile([64, 512], BF16, tag="ytb")
                            nc.vector.tensor_mul(
                                ytb[:], psOs[1][0:64, :], rbb[:])
                        nc.sync.dma_start(yT_t[64:128, pc, :], ytb[:])
                    # stage 4: out[t, co] partial for this tq block
                    for sub in range(4):
                        for nb2 in range(2):
                            ps4 = ps512.tile([128, 512], F32, tag="ps512")
                            for jc in range(4):
                                nc.tensor.matmul(
                                    ps4[:],
                                    yT_t[:, jc, sub * 128:(sub + 1) * 128],
                                    wp_sb[:, jc, nb2 * 512:(nb2 + 1) * 512],
                                    start=(jc == 0), stop=(jc == 3))
                            ot = outp.tile([128, 512], F32, tag="ot")
                            nc.vector.tensor_copy(ot[:], ps4[:])
                            t0 = (tqb * 4 + sub) * 128
                            nc.sync.dma_start(
                                yout[t0:t0 + 128, nb2 * 512:(nb2 + 1) * 512],
                                ot[:])


def build_nc(T=2048, reps=1):
    nc = bacc.Bacc()
    dram = dict(
        xT=nc.dram_tensor("xT", [C, T], F32R, kind="ExternalInput"),
        wqkT=nc.dram_tensor("wqkT", [C, 1024], F32R, kind="ExternalInput"),
        wvT=nc.dram_tensor("wvT", [C, JV], F32R, kind="ExternalInput"),
        wpTb=nc.dram_tensor("wpTb", [JV, C], BF16, kind="ExternalInput"),
        tri=nc.dram_tensor("tri", [128, 128], BF16, kind="ExternalInput"),
        ones64=nc.dram_tensor("ones64", [1, 64], F32R, kind="ExternalInput"),
        yout=nc.dram_tensor("yout", [T, C], F32, kind="ExternalOutput"),
    )
    with tile.TileContext(nc) as tc:
        for _ in range(reps):
            emit_body(nc, tc, dram, T)
    nc.compile()
    return nc


def shard_inputs(x, W_attn, W_proj, T):
    """Full inputs -> list of 8 per-core in_maps."""
    x = np.asarray(x, dtype=np.float32)
    W_attn = np.asarray(W_attn, dtype=np.float32)
    W_proj = np.asarray(W_proj, dtype=np.float32)

    p = np.arange(128)[:, None]
    c = np.arange(128)[None, :]
    tri = (c >= p).astype(ml_dtypes.bfloat16)

    in_maps = []
    for core in range(8):
        b, hg = core // 2, core % 2
        heads = [hg * HPC + i for i in range(HPC)]
        cols = []
        for pc in range(4):
            ha, hb = heads[2 * pc], heads[2 * pc + 1]
            cols += list(range(ha * 192, ha * 192 + 64))        # Q_a
            cols += list(range(hb * 192, hb * 192 + 64))        # Q_b
            cols += list(range(ha * 192 + 64, ha * 192 + 128))  # K_a
            cols += list(range(hb * 192 + 64, hb * 192 + 128))  # K_b
        vrows = [h * 192 + 128 + d for h in heads for d in range(64)]
        in_maps.append(dict(
            xT=np.ascontiguousarray(x[b, :T].T),
            wqkT=np.ascontiguousarray(W_attn[cols].T),
            wvT=np.ascontiguousarray(W_attn[vrows].T),
            tri=tri,
            ones64=np.ones((1, 64), dtype=np.float32),
            wpTb=np.ascontiguousarray(
                W_proj[:, hg * JV:(hg + 1) * JV].T).astype(ml_dtypes.bfloat16),
        ))
    return in_maps


def gather_outputs(results, T):
    out = np.empty((B, T, C), dtype=np.float32)
    for b in range(B):
        out[b] = results[2 * b]["yout"] + results[2 * b + 1]["yout"]
    return out


_NC_CACHE = {}


def run(x, W_attn, W_proj, T=2048, trace=False):
    if T not in _NC_CACHE:
        _NC_CACHE[T] = build_nc(T)
    nc = _NC_CACHE[T]
    in_maps = shard_inputs(x, W_attn, W_proj, T)
    res = run_bass_kernel_spmd(nc, in_maps, core_ids=list(range(8)), trace=trace)
    return gather_outputs(res.results, T), res


def kernel(x, W_attn, W_proj):
    out, _ = run(x, W_attn, W_proj, T=2048)
    return out
